# revision 16
# baseline (speedup 1.0000x reference)
"""BinaryLinear (sign-binarized weight linear layer) on 8 Trainium2 NeuronCores.

y[b,s,o] = sum_i x[b,s,i] * (scale[o] * sign(w[o,i])) + bias[o]
  with scale[o] = mean_i |w[o,i]|

Sharding: data-parallel over the batch dim (8 batches -> 8 cores); w/bias
replicated. Host passes x and w pre-cast to bf16 (the device matmul is bf16
either way; sign/scale/matmul/bias all stay on device). Per core:

  - x^T built by 4 big XBAR DMA-transposes straight from the bf16 input
    (one per 512-column chunk, full 8 KB source rows), resident in SBUF
  - w binarized on-chip: ACT sign -> bf16 B rows, DVE abs-row-mean -> scale;
    B written to DRAM (gpsimd) and XBAR-transposed back as B^T tiles in
    GROUPS of 2 o-blocks (amortizes the ~5us per-transpose sem latency)
  - ALL DMA_TRANSPOSEs are issued on the sync queue: two concurrent XBAR
    transposes on different HWDGE rings corrupt each other (HW-verified)
  - TensorE: yT[o,m] = B^T.T @ x^T accumulated over k in PSUM; DVE fuses
    psum*scale[o]+bias[o] on eviction; yT writes go out on scalar
  - two passes over the o-blocks: pass 1 computes n in {0,1}, pass 2
    n in {2,3}; B^T groups are re-transposed for pass 2 (B^T can't stay
    resident in SBUF next to the 128 KiB/partition x^T). This keeps the
    per-group cadence (28.8us) comfortably above the group transpose
    cost (~14us) with no per-block re-streams.

Host side only casts/shards inputs and transposes yT shards back into y.
"""

import numpy as np

B_DIM = 8
S_DIM = 2048
IN_F = 4096
OUT_F = 4096
P = 128
N_CORES = 8
N_TILE = 512
G = 2  # o-blocks per B^T transpose group

_BUILT = None


def _build_nc(s_dim=S_DIM, in_f=IN_F, out_f=OUT_F):
    from contextlib import ExitStack

    import concourse.mybir as mybir
    import concourse.tile as tile
    from concourse import bacc
    from concourse.bass import ts

    f32 = mybir.dt.float32
    bf16 = mybir.dt.bfloat16

    NCH = s_dim // N_TILE  # n chunks (moving-dim tiles of 512)
    PO = out_f // P  # o blocks (output-partition tiles of 128)
    KT = in_f // P  # contraction subtiles of 128
    NG = PO // G  # B^T groups
    HALF = in_f // 2
    NPASS = 2
    assert NCH % NPASS == 0
    NPC = NCH // NPASS  # chunks consumed per pass

    nc = bacc.Bacc(None, target_bir_lowering=False, debug=False)
    with tile.TileContext(nc) as tc:
        x_d = nc.dram_tensor("x", (s_dim, in_f), bf16, kind="ExternalInput")
        w_d = nc.dram_tensor("w", (out_f, in_f), bf16, kind="ExternalInput")
        b_d = nc.dram_tensor("bias", (out_f,), f32, kind="ExternalInput")
        yT_d = nc.dram_tensor("yT", (out_f, s_dim), f32, kind="ExternalOutput")

        with ExitStack() as ctx:
            dram = ctx.enter_context(tc.tile_pool(name="dram", bufs=1, space="DRAM"))
            bw_d = dram.tile((out_f, in_f), bf16)
            yT3 = yT_d[:, :].rearrange("(po pi) s -> pi po s", pi=P)

            const = ctx.enter_context(tc.tile_pool(name="const", bufs=1))
            # one resident x^T tile PER chunk: a single big tile would make
            # every matmul wait on ALL previously-emitted chunk transposes
            # (tile-granular dependency tracking)
            xTs = [
                const.tile([P, KT, N_TILE], bf16, name=f"xT{c}")
                for c in range(NCH)
            ]
            scale_sb = const.tile([P, PO], f32)
            bias_sb = const.tile([P, PO], f32)
            nc.scalar.dma_start(bias_sb[:], b_d[:].rearrange("(po pi) -> pi po", pi=P))

            wpool = ctx.enter_context(tc.tile_pool(name="wpool", bufs=2))
            bpool = ctx.enter_context(tc.tile_pool(name="bpool", bufs=2))
            scpool = ctx.enter_context(tc.tile_pool(name="scpool", bufs=2))
            btpool = ctx.enter_context(tc.tile_pool(name="btpool", bufs=3))
            opool = ctx.enter_context(tc.tile_pool(name="opool", bufs=4))
            psum = ctx.enter_context(tc.tile_pool(name="psum", bufs=6, space="PSUM"))

            def T_chunk(c):
                # 4 MB XBAR transpose: x rows [512c, 512c+512) -> xTs[c]
                nc.sync.dma_start_transpose(xTs[c][:], x_d[ts(c, N_TILE), :])

            w_tiles = {}

            def load_w(m):
                halves = []
                for h in range(2):
                    w_sb = wpool.tile([P, HALF], bf16, tag="w", name=f"w_{m}_{h}")
                    # SWDGE: keeps W loads off the HWDGE rings, which
                    # serialize against in-flight XBAR transposes
                    nc.gpsimd.dma_start(w_sb[:], w_d[ts(m, P), ts(h, HALF)])
                    halves.append(w_sb)
                w_tiles[m] = halves

            def process_w(m):
                # sign -> bf16 B rows (ACT), |w| row sums -> scale (DVE),
                # B rows -> DRAM (gpsimd SWDGE, off the HWDGE rings)
                sc2 = scpool.tile([P, 2], f32)
                for h in range(2):
                    w_sb = w_tiles[m][h]
                    b_sb = bpool.tile([P, HALF], bf16)
                    nc.scalar.sign(b_sb[:], w_sb[:])
                    nc.vector.tensor_reduce(
                        sc2[:, h : h + 1],
                        w_sb[:],
                        axis=mybir.AxisListType.X,
                        op=mybir.AluOpType.add,
                        apply_absolute_value=True,
                    )
                    nc.gpsimd.dma_start(bw_d[ts(m, P), ts(h, HALF)], b_sb[:])
                del w_tiles[m]
                nc.vector.tensor_reduce(
                    scale_sb[:, m : m + 1],
                    sc2[:],
                    axis=mybir.AxisListType.X,
                    op=mybir.AluOpType.add,
                )
                nc.vector.tensor_scalar_mul(
                    scale_sb[:, m : m + 1], scale_sb[:, m : m + 1], 1.0 / in_f
                )

            def load_btg(g):
                # one XBAR transpose covering G consecutive o-blocks:
                # bw rows [G*P*g, G*P*(g+1)) -> [128, KT, G*128]
                bt = btpool.tile([P, KT, G * P], bf16)
                nc.sync.dma_start_transpose(bt[:], bw_d[ts(g, G * P), :])
                return bt

            def mm_block(btg, j, m, n):
                ps = psum.tile([P, N_TILE], f32, name="ps")
                for kt in range(KT):
                    nc.tensor.matmul(
                        ps[:],
                        btg[:, kt, ts(j, P)],
                        xTs[n][:, kt, :],
                        start=(kt == 0),
                        stop=(kt == KT - 1),
                    )
                ob = opool.tile([P, N_TILE], f32)
                nc.vector.tensor_scalar(
                    ob[:],
                    ps[:],
                    scale_sb[:, m : m + 1],
                    bias_sb[:, m : m + 1],
                    op0=mybir.AluOpType.mult,
                    op1=mybir.AluOpType.add,
                )
                nc.scalar.dma_start(yT3[:, m, ts(n, N_TILE)], ob[:])

            # W-prep prefetch for the first two groups, then the XBAR
            # pipeline: T(chunk0), btg0, T(chunk1), btg1, ... all serial on
            # sync. PE starts once T0+btg0 land (~50us).
            next_proc = 0

            def advance_prep(k=1):
                nonlocal next_proc
                for _ in range(k):
                    if next_proc < PO:
                        load_w(next_proc)
                        process_w(next_proc)
                        next_proc += 1

            advance_prep(3 * G)
            T_chunk(0)
            bt_next = load_btg(0)
            T_chunk(1)

            emitted_T = 2
            for p in range(NPASS):
                for g in range(NG):
                    btg = bt_next
                    # prefetch the next group's transpose (next pass's group
                    # 0 is prefetched at the tail of this pass)
                    if g + 1 < NG:
                        advance_prep(G)
                        bt_next = load_btg(g + 1)
                    elif p + 1 < NPASS:
                        bt_next = load_btg(0)
                    for n in range(p * NPC, (p + 1) * NPC):
                        for j in range(G):
                            mm_block(btg, j, g * G + j, n)
                    # pass-2 chunks, once the early-group crunch is past
                    if p == 0 and g in (2, 4) and emitted_T < NCH:
                        T_chunk(emitted_T)
                        emitted_T += 1
    nc.finalize()
    return nc


def _get_nc():
    global _BUILT
    if _BUILT is None:
        _BUILT = _build_nc()
    return _BUILT


def kernel(x, weight, bias):
    import ml_dtypes
    from concourse.bass_utils import run_bass_kernel_spmd

    x = np.asarray(x)
    weight = np.asarray(weight)
    bias = np.asarray(bias, dtype=np.float32)
    assert x.shape == (B_DIM, S_DIM, IN_F), x.shape

    x_bf = np.ascontiguousarray(x).astype(ml_dtypes.bfloat16)
    w_bf = np.ascontiguousarray(weight).astype(ml_dtypes.bfloat16)

    nc = _get_nc()
    in_maps = [
        {"x": np.ascontiguousarray(x_bf[b]), "w": w_bf, "bias": bias}
        for b in range(N_CORES)
    ]
    res = run_bass_kernel_spmd(nc, in_maps, core_ids=list(range(N_CORES)))
    out = np.empty((B_DIM, S_DIM, OUT_F), dtype=np.float32)
    for b in range(N_CORES):
        out[b] = res.results[b]["yT"].T
    return out


# revision 17
# speedup vs baseline: 1.0508x; 1.0508x over previous
"""BinaryLinear (sign-binarized weight linear layer) on 8 Trainium2 NeuronCores.

y[b,s,o] = sum_i x[b,s,i] * (scale[o] * sign(w[o,i])) + bias[o]
  with scale[o] = mean_i |w[o,i]|

Sharding: data-parallel over the batch dim (8 batches -> 8 cores); w/bias
replicated. Host passes x and w pre-cast to bf16 (the device matmul is bf16
either way; sign/scale/matmul/bias all stay on device). Per core:

  - x^T built by 4 big XBAR DMA-transposes straight from the bf16 input
    (one per 512-column chunk, full 8 KB source rows), resident in SBUF
  - w binarized on-chip: ACT sign -> bf16 B rows, DVE abs-row-mean -> scale;
    B written to DRAM (gpsimd) and XBAR-transposed back as B^T tiles in
    GROUPS of 2 o-blocks (amortizes the ~5us per-transpose sem latency)
  - ALL DMA_TRANSPOSEs are issued on the sync queue: two concurrent XBAR
    transposes on different HWDGE rings corrupt each other (HW-verified)
  - TensorE: yT[o,m] = B^T.T @ x^T accumulated over k in PSUM; DVE fuses
    psum*scale[o]+bias[o] on eviction; yT writes go out on scalar
  - XBAR transposes exclude ALL other DMA traffic (HWDGE and SWDGE)
    while in flight, so W-prep for the first groups runs before the
    first transpose, and each group's DMA (B^T transpose + W loads +
    B writes + yT writes, ~31us) fits in half its 57.6us compute
  - single pass, block-major: each group of 2 o-blocks computes all 4
    n-chunks back-to-back, so every B^T group is transposed exactly once

Host side only casts/shards inputs and transposes yT shards back into y.
"""

import numpy as np

B_DIM = 8
S_DIM = 2048
IN_F = 4096
OUT_F = 4096
P = 128
N_CORES = 8
N_TILE = 512
G = 2  # o-blocks per B^T transpose group

_BUILT = None


def _build_nc(s_dim=S_DIM, in_f=IN_F, out_f=OUT_F):
    from contextlib import ExitStack

    import concourse.mybir as mybir
    import concourse.tile as tile
    from concourse import bacc
    from concourse.bass import ts

    f32 = mybir.dt.float32
    bf16 = mybir.dt.bfloat16

    NCH = s_dim // N_TILE  # n chunks (moving-dim tiles of 512)
    PO = out_f // P  # o blocks (output-partition tiles of 128)
    KT = in_f // P  # contraction subtiles of 128
    NG = PO // G  # B^T groups
    HALF = in_f // 2

    nc = bacc.Bacc(None, target_bir_lowering=False, debug=False)
    with tile.TileContext(nc) as tc:
        x_d = nc.dram_tensor("x", (s_dim, in_f), bf16, kind="ExternalInput")
        w_d = nc.dram_tensor("w", (out_f, in_f), bf16, kind="ExternalInput")
        b_d = nc.dram_tensor("bias", (out_f,), f32, kind="ExternalInput")
        yT_d = nc.dram_tensor("yT", (out_f, s_dim), f32, kind="ExternalOutput")

        with ExitStack() as ctx:
            dram = ctx.enter_context(tc.tile_pool(name="dram", bufs=1, space="DRAM"))
            bw_d = dram.tile((out_f, in_f), bf16)
            yT3 = yT_d[:, :].rearrange("(po pi) s -> pi po s", pi=P)

            const = ctx.enter_context(tc.tile_pool(name="const", bufs=1))
            # one resident x^T tile PER chunk: a single big tile would make
            # every matmul wait on ALL previously-emitted chunk transposes
            # (tile-granular dependency tracking)
            xTs = [
                const.tile([P, KT, N_TILE], bf16, name=f"xT{c}")
                for c in range(NCH)
            ]
            scale_sb = const.tile([P, PO], f32)
            bias_sb = const.tile([P, PO], f32)
            nc.scalar.dma_start(bias_sb[:], b_d[:].rearrange("(po pi) -> pi po", pi=P))

            wpool = ctx.enter_context(tc.tile_pool(name="wpool", bufs=2))
            bpool = ctx.enter_context(tc.tile_pool(name="bpool", bufs=2))
            scpool = ctx.enter_context(tc.tile_pool(name="scpool", bufs=2))
            btpool = ctx.enter_context(tc.tile_pool(name="btpool", bufs=3))
            opool = ctx.enter_context(tc.tile_pool(name="opool", bufs=4))
            psum = ctx.enter_context(tc.tile_pool(name="psum", bufs=6, space="PSUM"))

            def T_chunk(c):
                # 4 MB XBAR transpose: x rows [512c, 512c+512) -> xTs[c]
                nc.sync.dma_start_transpose(xTs[c][:], x_d[ts(c, N_TILE), :])

            w_tiles = {}

            def load_w(m):
                halves = []
                for h in range(2):
                    w_sb = wpool.tile([P, HALF], bf16, tag="w", name=f"w_{m}_{h}")
                    # SWDGE: keeps W loads off the HWDGE rings, which
                    # serialize against in-flight XBAR transposes
                    nc.gpsimd.dma_start(w_sb[:], w_d[ts(m, P), ts(h, HALF)])
                    halves.append(w_sb)
                w_tiles[m] = halves

            def process_w(m):
                # sign -> bf16 B rows (ACT), |w| row sums -> scale (DVE),
                # B rows -> DRAM (gpsimd SWDGE, off the HWDGE rings)
                sc2 = scpool.tile([P, 2], f32)
                for h in range(2):
                    w_sb = w_tiles[m][h]
                    b_sb = bpool.tile([P, HALF], bf16)
                    nc.scalar.sign(b_sb[:], w_sb[:])
                    nc.vector.tensor_reduce(
                        sc2[:, h : h + 1],
                        w_sb[:],
                        axis=mybir.AxisListType.X,
                        op=mybir.AluOpType.add,
                        apply_absolute_value=True,
                    )
                    nc.gpsimd.dma_start(bw_d[ts(m, P), ts(h, HALF)], b_sb[:])
                del w_tiles[m]
                nc.vector.tensor_reduce(
                    scale_sb[:, m : m + 1],
                    sc2[:],
                    axis=mybir.AxisListType.X,
                    op=mybir.AluOpType.add,
                )
                nc.vector.tensor_scalar_mul(
                    scale_sb[:, m : m + 1], scale_sb[:, m : m + 1], 1.0 / in_f
                )

            def load_btg(g):
                # one XBAR transpose covering G consecutive o-blocks:
                # bw rows [G*P*g, G*P*(g+1)) -> [128, KT, G*128]
                bt = btpool.tile([P, KT, G * P], bf16)
                nc.sync.dma_start_transpose(bt[:], bw_d[ts(g, G * P), :])
                return bt

            def mm_block(btg, j, m, n):
                ps = psum.tile([P, N_TILE], f32, name="ps")
                for kt in range(KT):
                    nc.tensor.matmul(
                        ps[:],
                        btg[:, kt, ts(j, P)],
                        xTs[n][:, kt, :],
                        start=(kt == 0),
                        stop=(kt == KT - 1),
                    )
                ob = opool.tile([P, N_TILE], f32)
                nc.vector.tensor_scalar(
                    ob[:],
                    ps[:],
                    scale_sb[:, m : m + 1],
                    bias_sb[:, m : m + 1],
                    op0=mybir.AluOpType.mult,
                    op1=mybir.AluOpType.add,
                )
                nc.scalar.dma_start(yT3[:, m, ts(n, N_TILE)], ob[:])

            # W-prep for the first two groups runs BEFORE any transpose
            # (transposes stall all other DMA); then the sync queue runs
            # T0, btg0, T1..T3, and one btg prefetch per group.
            next_proc = 0

            def advance_prep(k=1):
                nonlocal next_proc
                for _ in range(k):
                    if next_proc < PO:
                        load_w(next_proc)
                        process_w(next_proc)
                        next_proc += 1

            advance_prep(2 * G)
            T_chunk(0)
            bt_next = load_btg(0)
            for c in range(1, NCH):
                T_chunk(c)

            for g in range(NG):
                btg = bt_next
                if g + 1 < NG:
                    advance_prep(G)
                    bt_next = load_btg(g + 1)
                for n in range(NCH):
                    for j in range(G):
                        mm_block(btg, j, g * G + j, n)
    nc.finalize()
    return nc


def _get_nc():
    global _BUILT
    if _BUILT is None:
        _BUILT = _build_nc()
    return _BUILT


def kernel(x, weight, bias):
    import ml_dtypes
    from concourse.bass_utils import run_bass_kernel_spmd

    x = np.asarray(x)
    weight = np.asarray(weight)
    bias = np.asarray(bias, dtype=np.float32)
    assert x.shape == (B_DIM, S_DIM, IN_F), x.shape

    x_bf = np.ascontiguousarray(x).astype(ml_dtypes.bfloat16)
    w_bf = np.ascontiguousarray(weight).astype(ml_dtypes.bfloat16)

    nc = _get_nc()
    in_maps = [
        {"x": np.ascontiguousarray(x_bf[b]), "w": w_bf, "bias": bias}
        for b in range(N_CORES)
    ]
    res = run_bass_kernel_spmd(nc, in_maps, core_ids=list(range(N_CORES)))
    out = np.empty((B_DIM, S_DIM, OUT_F), dtype=np.float32)
    for b in range(N_CORES):
        out[b] = res.results[b]["yT"].T
    return out
